# revision 1
# baseline (speedup 1.0000x reference)
"""DBOW embedding-lookup kernel for Trainium2 (8 NeuronCores, SPMD).

Computes scores[b, k] = dot(D[doc_ids[b]], O[:, target_noise_ids[b, k]])
for B=16384, K=26, V=128, over doc table D [1e6, 128] f32 and word table
O [128, 1e5] f32.

Strategy: data-parallel over batch (2048 rows per core, 16 tiles of 128).
Host transposes O once to OT [1e5, 128] and casts it to bf16 (256B rows),
and pre-transposes the id arrays to [partition, tile] layout so all ids
live in SBUF as one resident tile. Per core: one doc-vector phase (16
single-index-per-partition indirect DMAs with f32->bf16 in-DMA cast into
a resident [128, 2048] bf16 tile), then per tile 26 single-index
w-gathers (bf16 word rows), a bf16 broadcast multiply on DVE (2x mode),
and an f32 reduce along the word dim.

Manual semaphores (no Tile framework): Tile throttles Pool-engine DMAs to
8 in-flight via per-lane completion waits, putting the ~2us DMA
completion receipt on the Pool critical path for each of the 432 gathers.
Here the gathers stream back-to-back on the Pool sequencer; completions
tick single counting semaphores (sound because each SDMA engine drains
its ring FIFO, so a counter threshold implies prefix completion). DVE
waits on cumulative counts; Pool's only back-pressure is one wait per
tile on the multiply counter (w-ring WAR, 8 tiles deep). The remaining
wall is the Q7 SWDGE indirect-descriptor emission rate (~11ns/descriptor
x 55k gathered rows per core).
"""

import numpy as np
import ml_dtypes

import concourse.bass as bass
import concourse.mybir as mybir
from concourse.bass_utils import run_bass_kernel_spmd
from concourse.library_overlay import lower_extended_insts


# --- compat shims for the walrus build in this container ---------------------
# 1) clear_and_free_semaphores emits EVENT_SEMAPHORE_RANGE_CLEAR + a
#    multi-wait Drain; this walrus rejects both encodings. With a single
#    context per program the freed sems are never reused, so the cleanup
#    instructions are dead weight — keep only the bookkeeping.
def _patched_clear_and_free(self, sems):
    if not sems:
        return
    sem_nums = [s.num if hasattr(s, "num") else s for s in sems]
    self._state.prepend_free_semaphores(sem_nums)
    for ps in self._tile_sem_poison_stack:
        ps.update(sem_nums)


bass.Bass.clear_and_free_semaphores = _patched_clear_and_free


# 2) This walrus encodes at most ONE sync-wait per instruction; block exits
#    can attach several. Split the extras into wait-only NoOps on the same
#    engine just before the instruction (same-engine program order preserves
#    semantics).
def _split_multi_waits(nc):
    n_new = 0
    for f in nc.m.functions:
        for bb in f.blocks:
            out = []
            changed = False
            for inst in bb.instructions:
                si = inst.sync_info
                waits = list(si.on_wait) if si is not None and si.on_wait else []
                if len(waits) > 1:
                    changed = True
                    for w in waits[:-1]:
                        nop = mybir.InstNoOp(
                            name=f"{inst.name}_w{n_new}", ins=[], outs=[]
                        )
                        n_new += 1
                        nop.engine = inst.engine
                        nop.sync_info = mybir.SyncInfo(on_wait=[w], on_update=[])
                        out.append(nop)
                    inst.sync_info = mybir.SyncInfo(
                        on_wait=[waits[-1]],
                        on_update=list(si.on_update) if si.on_update else [],
                    )
                out.append(inst)
            if changed:
                bb.instructions = out
    return n_new


VEC = 128
NUM_DOCS = 1_000_000
NUM_WORDS = 100_000
BATCH = 16_384
K = 26
NCORES = 8
BLOC = BATCH // NCORES          # 2048 batch rows per core
NTILES = BLOC // 128            # 16 tiles of 128 rows
KV = K * VEC
B = 8        # w ring depth (tiles)
SC = 4       # scores ring depth

F32 = mybir.dt.float32
BF16 = mybir.dt.bfloat16
I32 = mybir.dt.int32

_cached = {}


def _build_program(repeat=1, mode="full"):
    nc = bass.Bass(trn_type="TRN2")

    D_t = nc.dram_tensor("D", [NUM_DOCS, VEC], F32, kind="ExternalInput")
    OT_t = nc.dram_tensor("OT", [NUM_WORDS, VEC], BF16, kind="ExternalInput")
    did_t = nc.dram_tensor("did_T", [128, NTILES], I32, kind="ExternalInput")
    nid_t = nc.dram_tensor("nid_T", [128, NTILES * K], I32, kind="ExternalInput")
    out_t = nc.dram_tensor("out", [BLOC, K], F32, kind="ExternalOutput")

    s_ids = nc.alloc_semaphore("s_ids")
    s_d = nc.alloc_semaphore("s_d")
    s_w = nc.alloc_semaphore("s_w")
    s_mult = nc.alloc_semaphore("s_mult")
    s_red = nc.alloc_semaphore("s_red")
    s_out = nc.alloc_semaphore("s_out")

    N = repeat * NTILES

    with (
        nc.sbuf_tensor([128, NTILES * K], I32) as nid,
        nc.sbuf_tensor([128, NTILES], I32) as did,
        nc.sbuf_tensor([128, NTILES * VEC], BF16) as d_all,
        nc.sbuf_tensor([128, B * KV], BF16) as wring,
        nc.sbuf_tensor([128, KV], BF16) as prod,
        nc.sbuf_tensor([128, SC * K], F32) as scring,
        nc.Block(),
    ):
        # --- sync (SP, HWDGE): id loads ------------------------------------
        nc.sync.dma_start(out=nid[:], in_=nid_t[:, :]).then_inc(s_ids, 16)
        nc.sync.dma_start(out=did[:], in_=did_t[:, :]).then_inc(s_ids, 16)

        # --- Pool (SWDGE): doc gathers, then word gathers ------------------
        nc.gpsimd.wait_ge(s_ids, 32)
        for t in range(NTILES):
            nc.gpsimd.indirect_dma_start(
                out=d_all[:, t * VEC : (t + 1) * VEC],
                out_offset=None,
                in_=D_t[:],
                in_offset=bass.IndirectOffsetOnAxis(ap=did[:, t : t + 1], axis=0),
            ).then_inc(s_d, 16)
        for n in range(N):
            t = n % NTILES
            slot = n % B
            if n >= B and mode != "gather":
                nc.gpsimd.wait_ge(s_mult, n - B + 1)
            for k in range(K):
                nc.gpsimd.indirect_dma_start(
                    out=wring[:, slot * KV + k * VEC : slot * KV + (k + 1) * VEC],
                    out_offset=None,
                    in_=OT_t[:],
                    in_offset=bass.IndirectOffsetOnAxis(
                        ap=nid[:, t * K + k : t * K + k + 1], axis=0
                    ),
                ).then_inc(s_w, 16)

        # --- DVE: multiply + reduce; sync: stores --------------------------
        if mode != "gather":
            nc.vector.wait_ge(s_d, 16 * NTILES)
            for n in range(N):
                t = n % NTILES
                slot = n % B
                ssl = n % SC
                nc.vector.wait_ge(s_w, 16 * K * (n + 1))
                d3 = (
                    d_all[:, t * VEC : (t + 1) * VEC]
                    .unsqueeze(1)
                    .broadcast_to([128, K, VEC])
                )
                wv = wring[:, slot * KV : (slot + 1) * KV]
                nc.vector.tensor_tensor(
                    out=prod[:].rearrange("p (k v) -> p k v", v=VEC),
                    in0=wv.rearrange("p (k v) -> p k v", v=VEC),
                    in1=d3,
                    op=mybir.AluOpType.mult,
                ).then_inc(s_mult, 1)
                if n >= SC:
                    nc.vector.wait_ge(s_out, 16 * (n - SC + 1))
                nc.vector.tensor_reduce(
                    out=scring[:, ssl * K : (ssl + 1) * K],
                    in_=prod[:].rearrange("p (k v) -> p k v", v=VEC),
                    axis=mybir.AxisListType.X,
                    op=mybir.AluOpType.add,
                ).then_inc(s_red, 1)

            for n in range(N):
                t = n % NTILES
                ssl = n % SC
                nc.sync.wait_ge(s_red, n + 1)
                nc.sync.dma_start(
                    out=out_t[t * 128 : (t + 1) * 128, :],
                    in_=scring[:, ssl * K : (ssl + 1) * K],
                ).then_inc(s_out, 16)
            nc.sync.wait_ge(s_out, 16 * N)
        else:
            nc.vector.wait_ge(s_w, 16 * K * N)

    _split_multi_waits(nc)
    lower_extended_insts(nc)
    return nc


def _get_program(repeat=1, mode="full"):
    key = (repeat, mode)
    if key not in _cached:
        _cached[key] = _build_program(repeat, mode)
    return _cached[key]


_host_cache = {}


def _make_in_maps(context_ids, doc_ids, target_noise_ids, D, O):
    D = np.ascontiguousarray(np.asarray(D, dtype=np.float32))
    okey = id(O)
    if _host_cache.get("okey") != okey:
        _host_cache["okey"] = okey
        _host_cache["OT"] = np.ascontiguousarray(
            np.asarray(O, dtype=np.float32).T.astype(ml_dtypes.bfloat16)
        )
    OT = _host_cache["OT"]
    doc_ids = np.asarray(doc_ids, dtype=np.int32)
    noise = np.asarray(target_noise_ids, dtype=np.int32)

    in_maps = []
    for c in range(NCORES):
        sl = slice(c * BLOC, (c + 1) * BLOC)
        # [p, t] / [p, t*K+k] layouts: element (p, t, k) = row t*128+p of slice
        did_T = np.ascontiguousarray(doc_ids[sl].reshape(NTILES, 128).T)
        nid_T = np.ascontiguousarray(
            noise[sl].reshape(NTILES, 128, K).transpose(1, 0, 2).reshape(128, NTILES * K)
        )
        in_maps.append({"D": D, "OT": OT, "did_T": did_T, "nid_T": nid_T})
    return in_maps


def run(inputs, trace=False, repeat=1, mode="full", **kw):
    """Run the SPMD kernel; returns (full_output, BassKernelResults)."""
    nc = _get_program(repeat, mode)
    in_maps = _make_in_maps(**inputs)
    res = run_bass_kernel_spmd(
        nc, in_maps, core_ids=list(range(NCORES)), trace=trace, **kw
    )
    out = np.concatenate([r["out"] for r in res.results], axis=0)
    return out, res


def kernel(**inputs):
    out, _ = run(inputs, trace=False)
    return out

